# revision 10
# baseline (speedup 1.0000x reference)
"""Distributed Trainium2 (Bass/Tile) kernel: supervised contrastive loss.

reference semantics (jax, fp32):
    sim    = (E @ E.T) / 0.07                  # [N, N]
    exps   = exp(sim) * (1 - eye)              # overflow -> inf, inf*0 -> nan
    mask   = (labels[None,:] == labels[:,None]) * (1 - eye)
    pos    = sum(exps * mask, axis=1)
    neg    = sum(exps, axis=1) - pos
    loss   = mean(-log(pos / (pos + neg + eps) + eps))

Sharding: rows are split into 8 blocks of 1024 (one per NeuronCore). Each
core receives a *rolled* copy of the embeddings (its own row block first)
transposed to [D, N] so the contraction dim sits on SBUF partitions, and
computes the transposed similarity block simT[j, i] = sim[j, rows_mine]
tile-by-tile on the TensorEngine (float32r, 1 cyc/row).

pos/total row sums are computed on the TensorEngine as label-class sums:
    C[c, i] = sum_j onehot(labels[j] == c) * exps[j, i]   (PSUM accum, 64 tiles)
    tot[i]  = sum_c C[c, i]          (ones-vector matmul)
    pos[i]  = C[labels[i], i]        (onehot-mask multiply + ones matmul)
which keeps the masked reduction off the VectorEngine. The diagonal is
zeroed post-exp by a mask multiply (a static pattern thanks to the roll;
inf*0 -> nan exactly as the reference's exp_sim * (1 - eye)). The per-row
loss vector is returned per core; the host gathers and takes the mean
(the final "all-reduce").
"""

import numpy as np

N, D, NCORES = 8192, 512, 8
R = N // NCORES          # 1024 rows per core
P = 128                  # SBUF partitions
NT = N // P              # 64 j-tiles
NG = 8                   # ET column chunks (1024 wide) per d-block
NIC = R // 512           # 2 i-chunks of 512
TEMP = 0.07
EPS = 1e-8

_CACHE = {}


def _build(scale: float, reps: int = 1):
    """Build + compile the (single, SPMD) Bass program. scale = 1/temperature."""
    import concourse.bacc as bacc
    import concourse.mybir as mybir
    import concourse.tile as tile

    dt = mybir.dt
    f32, f32r, bf16 = dt.float32, dt.float32r, dt.bfloat16
    AF = mybir.ActivationFunctionType
    ALU = mybir.AluOpType

    nc = bacc.Bacc("TRN2", target_bir_lowering=False, debug=False)
    embT = nc.dram_tensor("embT", [D, N], f32r, kind="ExternalInput").ap()
    dmask = nc.dram_tensor("dmask", [P, 4 * 512], bf16, kind="ExternalInput").ap()
    labels_pt = nc.dram_tensor("labels_pt", [P, NT], f32, kind="ExternalInput").ap()
    labels_mine = nc.dram_tensor("labels_mine", [P, R], f32, kind="ExternalInput").ap()
    c_iota = nc.dram_tensor("c_iota", [P, P], f32, kind="ExternalInput").ap()
    p_iota = nc.dram_tensor("p_iota", [P, 1], f32, kind="ExternalInput").ap()
    loss_out = nc.dram_tensor("loss", [1, R], f32, kind="ExternalOutput").ap()

    with tile.TileContext(nc) as tc:
        with (
            tc.tile_pool(name="et", bufs=1) as etp,
            tc.tile_pool(name="consts", bufs=1) as cp,
            tc.tile_pool(name="oh", bufs=3) as ohp,
            tc.tile_pool(name="exps", bufs=6) as exp_pool,
            tc.tile_pool(name="misc", bufs=1) as mp,
            tc.tile_pool(name="ps", bufs=4, space="PSUM") as psp,
            tc.tile_pool(name="cps", bufs=1, space="PSUM") as cpsp,
        ):
            # small constants
            lp = cp.tile([P, NT], f32, tag="lp", name="lp")
            nc.sync.dma_start(lp[:], labels_pt)
            lm = cp.tile([P, R], f32, tag="lm", name="lm")
            nc.sync.dma_start(lm[:], labels_mine)
            ci = cp.tile([P, P], f32, tag="ci", name="ci")
            nc.sync.dma_start(ci[:], c_iota)
            pi = cp.tile([P, 1], f32, tag="pi", name="pi")
            nc.sync.dma_start(pi[:], p_iota)
            ones = cp.tile([P, 1], bf16, tag="ones", name="ones")
            nc.vector.memset(ones[:], 1.0)
            epsb = cp.tile([P, 1], f32, tag="epsb", name="epsb")
            nc.vector.memset(epsb[:], EPS)
            dm = cp.tile([P, 4 * 512], bf16, tag="dm", name="dm")
            nc.sync.dma_start(dm[:], dmask)

            # ET resident in SBUF: 4 d-blocks x 8 chunks of [128, 1024] (16 MB).
            # Chunk g=0 (= this core's own columns / moving operand) loads first.
            et = [[None] * NG for _ in range(4)]
            for g in range(NG):
                for dd in range(4):
                    t_ = etp.tile([P, 1024], f32r, tag=f"et{dd}_{g}", name=f"et{dd}_{g}")
                    nc.sync.dma_start(
                        t_[:], embT[dd * 128 : (dd + 1) * 128, g * 1024 : (g + 1) * 1024]
                    )
                    et[dd][g] = t_

            for _rep in range(reps):
                _body(nc, tc, mybir, scale, et, lp, lm, ci, pi, ones, epsb, dm,
                      ohp, exp_pool, mp, psp, cpsp, loss_out)

    nc.compile()
    return nc


def _body(nc, tc, mybir, scale, et, lp, lm, ci, pi, ones, epsb, dm,
          ohp, exp_pool, mp, psp, cpsp, loss_out):
    dt = mybir.dt
    f32, bf16 = dt.float32, dt.bfloat16
    AF = mybir.ActivationFunctionType
    ALU = mybir.AluOpType
    if True:
        if True:
            # class-sum accumulators (held in PSUM across the whole j loop)
            c_ps = [cpsp.tile([P, 512], f32, tag=f"c{ic}", name=f"c{ic}") for ic in range(NIC)]

            # main loop over j-tiles; class matmul runs one step behind the
            # sim matmul so the PE never stalls on the ACT exp.
            prev = None
            for t in range(NT):
                g, jj = divmod(t, NG)
                oh_t = ohp.tile([P, P], bf16, tag="oh", name="oh")
                nc.vector.tensor_scalar(
                    oh_t[:], ci[:], lp[:, t : t + 1], None, op0=ALU.is_equal
                )
                exs = []
                for ic in range(NIC):
                    ps = psp.tile([P, 512], f32, tag="ps", name="ps")
                    for dd in range(4):
                        nc.tensor.matmul(
                            ps[:],
                            et[dd][g][:, jj * 128 : (jj + 1) * 128],
                            et[dd][0][:, ic * 512 : (ic + 1) * 512],
                            start=(dd == 0),
                            stop=(dd == 3),
                        )
                    ex = exp_pool.tile([P, 512], bf16, tag="ex", name="ex")
                    nc.scalar.activation(ex[:], ps[:], AF.Exp, scale=scale)
                    if t < 8 and ic == t // 4:
                        # zero this tile's diagonal (column f = t*128+p - ic*512)
                        # via mask multiply; inf*0 -> nan exactly as in the
                        # reference's exp_sim * (1 - eye).
                        q = t % 4
                        nc.vector.tensor_mul(
                            ex[:], ex[:], dm[:, q * 512 : (q + 1) * 512]
                        )
                    exs.append(ex)
                if prev is not None:
                    p_oh, p_exs, pt = prev
                    for ic in range(NIC):
                        nc.tensor.matmul(
                            c_ps[ic][:],
                            p_oh[:],
                            p_exs[ic][:],
                            start=(pt == 0),
                            stop=False,
                            skip_group_check=True,
                        )
                prev = (oh_t, exs, t)
            p_oh, p_exs, pt = prev
            for ic in range(NIC):
                nc.tensor.matmul(
                    c_ps[ic][:],
                    p_oh[:],
                    p_exs[ic][:],
                    start=False,
                    stop=True,
                    skip_group_check=True,
                )

            # epilogue: tot = sum_c C, pos = C[label_i], loss = -log(pos/(pos+neg+eps)+eps)
            loss_sb = mp.tile([1, R], f32, tag="loss", name="loss")
            for ic in range(NIC):
                csb = mp.tile([P, 512], bf16, tag=f"csb{ic}", name=f"csb{ic}")
                nc.vector.tensor_copy(csb[:], c_ps[ic][:])
                ohm = mp.tile([P, 512], bf16, tag=f"ohm{ic}", name=f"ohm{ic}")
                nc.vector.tensor_scalar(
                    ohm[:], lm[:, ic * 512 : (ic + 1) * 512], pi[:, 0:1], None,
                    op0=ALU.is_equal,
                )
                posm = mp.tile([P, 512], bf16, tag=f"posm{ic}", name=f"posm{ic}")
                nc.vector.tensor_mul(posm[:], csb[:], ohm[:])
                tot_ps = psp.tile([1, 512], f32, tag="ps", name="ps")
                nc.tensor.matmul(
                    tot_ps[:], ones[:], csb[:], start=True, stop=True,
                )
                pos_ps = psp.tile([1, 512], f32, tag="ps", name="ps")
                nc.tensor.matmul(
                    pos_ps[:], ones[:], posm[:], start=True, stop=True,
                )
                tot = mp.tile([1, 512], f32, tag=f"tot{ic}", name=f"tot{ic}")
                nc.vector.tensor_copy(tot[:], tot_ps[:])
                pos = mp.tile([1, 512], f32, tag=f"pos{ic}", name=f"pos{ic}")
                nc.vector.tensor_copy(pos[:], pos_ps[:])
                neg = mp.tile([1, 512], f32, tag=f"neg{ic}", name=f"neg{ic}")
                nc.vector.tensor_sub(neg[:], tot[:], pos[:])
                den = mp.tile([1, 512], f32, tag=f"den{ic}", name=f"den{ic}")
                nc.vector.scalar_tensor_tensor(
                    den[:], pos[:], EPS, neg[:], op0=ALU.add, op1=ALU.add
                )
                rec = mp.tile([1, 512], f32, tag=f"rec{ic}", name=f"rec{ic}")
                nc.vector.reciprocal(rec[:], den[:])
                rat = mp.tile([1, 512], f32, tag=f"rat{ic}", name=f"rat{ic}")
                nc.vector.tensor_mul(rat[:], pos[:], rec[:])
                lt = mp.tile([1, 512], f32, tag=f"lt{ic}", name=f"lt{ic}")
                nc.scalar.activation(lt[:], rat[:], AF.Ln, bias=epsb[0:1, 0:1], scale=1.0)
                nc.vector.tensor_scalar_mul(
                    loss_sb[:, ic * 512 : (ic + 1) * 512], lt[:], -1.0
                )
            nc.sync.dma_start(loss_out, loss_sb[:])


def _get_nc(scale: float, reps: int = 1):
    key = (scale, reps)
    if key not in _CACHE:
        _CACHE[key] = _build(scale, reps)
    return _CACHE[key]


def _prepare_in_maps(emb: np.ndarray, lab: np.ndarray):
    import ml_dtypes

    emb = np.ascontiguousarray(np.asarray(emb, dtype=np.float32))
    lab_f = np.asarray(lab).astype(np.float32)
    c_io = np.ascontiguousarray(
        np.tile(np.arange(P, dtype=np.float32)[None, :], (P, 1))
    )
    p_io = np.ascontiguousarray(np.arange(P, dtype=np.float32)[:, None])
    # 4 diagonal-complement mask tiles [128, 512]: pattern q zeroes f == q*128+p
    dmask = np.ones((P, 4 * 512), np.float32)
    for q in range(4):
        for p in range(P):
            dmask[p, q * 512 + q * 128 + p] = 0.0
    dmask = dmask.astype(ml_dtypes.bfloat16)
    maps = []
    for c in range(NCORES):
        er = np.roll(emb, -c * R, axis=0)
        lr = np.roll(lab_f, -c * R)
        maps.append(
            {
                "embT": np.ascontiguousarray(er.T),
                "labels_pt": np.ascontiguousarray(lr.reshape(NT, P).T),
                "labels_mine": np.ascontiguousarray(
                    np.broadcast_to(lr[:R][None, :], (P, R))
                ),
                "c_iota": c_io,
                "p_iota": p_io,
                "dmask": dmask,
            }
        )
    return maps


def _run(emb, lab, scale=1.0 / TEMP, trace=False, **spmd_kwargs):
    from concourse.bass_utils import run_bass_kernel_spmd

    nc = _get_nc(scale)
    in_maps = _prepare_in_maps(emb, lab)
    res = run_bass_kernel_spmd(
        nc, in_maps, list(range(NCORES)), trace=trace, **spmd_kwargs
    )
    loss = np.concatenate(
        [np.asarray(res.results[c]["loss"]).reshape(-1) for c in range(NCORES)]
    )
    return loss, res


def kernel(embeddings, labels) -> np.ndarray:
    loss, _ = _run(embeddings, labels)
    return np.asarray(np.mean(loss, dtype=np.float32), dtype=np.float32)
